# revision 75
# baseline (speedup 1.0000x reference)
"""Multi-head causal attention on 8 TRN2 NeuronCores.

Problem: x[4,2048,1024] @ Wqkv.T -> 16-head causal attention -> @ Wout.T.

Sharding: core c handles batch b=c//2, head-group g=c%2 (8 heads of 64).
Each core computes qkv for its (batch, head-group) slice, causal attention,
and a partial out-projection over its 512 columns of Wout's input dim.
Host sums the two partials per batch (the all-reduce of the hint).

Per-core layouts (host pre-transposes so every matmul contraction dim lands
on SBUF partitions):
  xT   [1024 d, 2048 t]      wqkT [1024 d, 1024 (q|k)e]
  wvT  [1024 d,  512 e]      woT  [ 512 e, 1024 f]
All tensors are fp16 (PSUM accumulation stays fp32): same 1-row/cycle PE
rate as fp32r but FWL halves LDWEIGHTS, DMA bytes halve, and the PE power
draw stays under the SW-throttle threshold that cost fp32r ~75us of K=4/8
clock-gating.  Simulated end-to-end fp16 error: 5.7e-4 rel (gate: 2e-2).

Schedule: a single flat software pipeline over (pair, j-block) iterations.
S for iteration i+1 is emitted before AV of iteration i (crossing pair
boundaries, so ACT never waits on a pair refill); next-chunk QKV production
groups and the previous chunk's out-projection blocks are sprinkled one
per iteration as PE filler; weights/x arrive via 5 consolidated strided
DMAs (sync-queue trigger rate, not bandwidth, gated the old prologue).
S head-pairs run concurrently on row-groups 0:63/64:127 (auto
tile_position from the 64-partition APs).
"""

import sys

sys.path.insert(0, "/opt/trn_rl_repo")

import numpy as np

B, T, D, H = 4, 2048, 1024, 16
E = 512  # per-core head width (8 heads x 64)
ND = 8  # d chunks of 128
NTC = 4  # t chunks of 512
SCALE = 0.125  # 1/sqrt(64)

_NC_CACHE = {}


def build():
    if "nc" in _NC_CACHE:
        return _NC_CACHE["nc"]
    import concourse.bacc as bacc
    import concourse.mybir as mybir
    import concourse.tile as tile

    F32 = mybir.dt.float32
    F16 = mybir.dt.float16
    EXP = mybir.ActivationFunctionType.Exp

    nc = bacc.Bacc("TRN2", target_bir_lowering=False, debug=False, num_devices=8)
    xT = nc.declare_dram_parameter("xT", [D, T], F16, isOutput=False)
    wqkT = nc.declare_dram_parameter("wqkT", [D, 2 * E], F16, isOutput=False)
    wvT = nc.declare_dram_parameter("wvT", [D, E], F16, isOutput=False)
    woT = nc.declare_dram_parameter("woT", [E, D], F16, isOutput=False)
    z = nc.declare_dram_parameter("z", [T, D], F16, isOutput=True)

    with tile.TileContext(nc) as tc:
        with (
            tc.tile_pool(name="pw", bufs=1) as pw,
            tc.tile_pool(name="px", bufs=3) as px,
            tc.tile_pool(name="pkt", bufs=4) as pkt,
            tc.tile_pool(name="pqt", bufs=8) as pqt,
            tc.tile_pool(name="pv", bufs=16) as pv,
            tc.tile_pool(name="ppt", bufs=2) as ppt,
            tc.tile_pool(name="pr", bufs=2) as pr,
            tc.tile_pool(name="pysb", bufs=8) as pysb,
            tc.tile_pool(name="pzsb", bufs=1) as pzsb,
            tc.tile_pool(name="pone", bufs=1) as pone,
            tc.tile_pool(name="ps", bufs=2, space="PSUM") as ps,
            tc.tile_pool(name="pyd", bufs=2, space="PSUM") as pyd,
        ):
            # ---- consolidated input DMAs (one trigger each; the sync
            # queue issues triggers at only ~0.65us apiece)
            wqk = pw.tile([128, ND * 2 * E], F16, tag="wqk")
            wqk3 = wqk[:].rearrange("p (dc e) -> p dc e", dc=ND)
            xs0 = px.tile([128, ND * 512], F16, tag="x", name="xs")
            wv = pw.tile([128, ND * E], F16, tag="wv")
            wo = pw.tile([128, 4 * D], F16, tag="wo")

            # transfers alternate across BOTH hwdge trigger queues
            # (SP + Activation) and arrive in order of first use: the host
            # stores wqkT columns as [q0,k0,q1,k1,...] so the first 512KB
            # chunk is exactly pair 0's q/k weights; the ~320GB/s aggregate
            # DMA bandwidth is the prologue's floor, so byte order is
            # everything
            xs03 = xs0[:].rearrange("p (dc t) -> p dc t", dc=ND)
            xT3 = xT[:, 0:512].rearrange("(dc p) t -> p dc t", p=128)
            wqkT3 = wqkT[:].rearrange("(dc p) e -> p dc e", p=128)
            wv3 = wv[:].rearrange("p (dc e) -> p dc e", dc=ND)
            wvT3 = wvT[:].rearrange("(dc p) e -> p dc e", p=128)
            nc.sync.dma_start(wqk3[:, :, 0:256], wqkT3[:, :, 0:256])
            nc.scalar.dma_start(xs03[:, 0:4, :], xT3[:, 0:4, :])
            nc.sync.dma_start(xs03[:, 4:8, :], xT3[:, 4:8, :])
            nc.scalar.dma_start(wv3[:, 0:4, :], wvT3[:, 0:4, :])
            nc.sync.dma_start(wv3[:, 4:8, :], wvT3[:, 4:8, :])
            nc.scalar.dma_start(wqk3[:, :, 256:512], wqkT3[:, :, 256:512])
            nc.sync.dma_start(wqk3[:, :, 512:768], wqkT3[:, :, 512:768])
            nc.scalar.dma_start(wqk3[:, :, 768:1024], wqkT3[:, :, 768:1024])
            nc.sync.dma_start(
                wo[:].rearrange("p (m f) -> p m f", m=4),
                woT[:].rearrange("(m p) f -> p m f", p=128),
            )

            # per-head filler block for the AV stationary: [ones(32)|zeros(32)]
            ones_f = pone.tile([128, 512], F16, tag="onef")
            of4 = ones_f[:].rearrange("p (hh c) -> p hh c", hh=8)
            nc.gpsimd.memset(of4[:, :, 0:32], 1.0)
            nc.gpsimd.memset(of4[:, :, 32:64], 0.0)

            # 0/1 upper-triangle (keep c>=j) mask for the causal window;
            # applied as a DVE multiply so gpsimd runs only its
            # partition_broadcast library (an affine_select/broadcast mix
            # thrashes the gpsimd custom-op library, ~6us per swap)
            mtri = pone.tile([128, 128], F16, tag="mtri")
            nc.vector.memset(mtri[:], 1.0)
            nc.gpsimd.affine_select(
                out=mtri[:],
                in_=mtri[:],
                compare_op=mybir.AluOpType.is_ge,
                fill=0.0,
                base=0,
                pattern=[[1, 128]],
                channel_multiplier=-1,
            )
            mtri3 = (
                mtri[:]
                .rearrange("p (o c) -> p o c", o=1)
                .broadcast_to((128, 2, 128))
            )

            # a few dummy matmuls on the ones tile bridge the PE from the
            # preamble to the first DMA arrival so the HAM clock-gate
            # warmup (~3.4us of sustained activity) starts immediately
            for _ in range(3):
                warm = ps.tile([128, 1024], F32, tag="st", name="warm")
                nc.tensor.matmul(
                    warm[:, 0:512],
                    ones_f[:, 0:128],
                    ones_f[:],
                    start=True,
                    stop=True,
                )

            # persistent K^T [e,t] tiles; pair m = heads 2m / 2m+1 at
            # partition rows 0:64 / 64:128
            kt = [
                pkt.tile([128, T], F16, tag="kt", name=f"kt{i}")
                for i in range(4)
            ]
            vt = [None] * 16  # V tiles per 128-row t-block

            def emit_x_load(tci):
                t_ = px.tile([128, ND * 512], F16, tag="x", name="xs")
                t0 = tci * 512
                nc.sync.dma_start(
                    t_[:].rearrange("p (dc t) -> p dc t", dc=ND),
                    xT[:, t0 : t0 + 512].rearrange(
                        "(dc p) t -> p dc t", p=128
                    ),
                )
                return t_

            def emit_qk_group(xs, m, tci):
                """m 0..3: Q chunk -> returns qt tile; 4..7: K chunk.
                wqk column order per dc is [q0,k0,q1,k1,...] (128 each)."""
                col = (m % 4) * 256 + (128 if m >= 4 else 0)
                acc = ps.tile([128, 1024], F32, tag="st", name="acc")
                acc = acc[:, 0:512]
                for dc in range(ND):
                    nc.tensor.matmul(
                        acc[:],
                        wqk[:, dc * 1024 + col : dc * 1024 + col + 128],
                        xs[:, dc * 512 : (dc + 1) * 512],
                        start=(dc == 0),
                        stop=(dc == ND - 1),
                    )
                if m < 4:
                    t_ = pqt.tile([128, 512], F16, tag="qt", name="qt")
                    nc.vector.tensor_copy(t_[:], acc[:])
                    return t_
                t0 = tci * 512
                nc.vector.tensor_copy(kt[m - 4][:, t0 : t0 + 512], acc[:])
                return None

            def emit_v_group(xs, tci, ts):
                """V tile layout per head: [V_h(64) | ones(32)] -> AV lhsT
                slices are 96 cols, putting Y at PSUM rows 0:64 and the
                denominator at 64:96."""
                jb = 4 * tci + ts
                acc = ps.tile([128, 1024], F32, tag="st", name="vacc")
                acc = acc[:, 0:512]
                for dc in range(ND):
                    nc.tensor.matmul(
                        acc[:],
                        xs[:, dc * 512 + ts * 128 : dc * 512 + (ts + 1) * 128],
                        wv[:, dc * 512 : (dc + 1) * 512],
                        start=(dc == 0),
                        stop=(dc == ND - 1),
                    )
                t_ = pv.tile([128, 768], F16, tag="v", name="vt")
                t4 = t_[:].rearrange("p (hh c) -> p hh c", hh=8)
                a4 = acc[:].rearrange("p (hh c) -> p hh c", hh=8)
                nc.vector.tensor_copy(t4[:, :, 0:64], a4[:])
                nc.vector.tensor_copy(t4[:, :, 64:96], of4[:, :, 0:32])
                vt[jb] = t_

            def emit_outproj_block(ysbs, tci, ib, fh):
                zp = ps.tile([128, 1024], F32, tag="st", name="zp")
                zp = zp[:, 0:512]
                for m in range(4):
                    nc.tensor.matmul(
                        zp[:],
                        ysbs[m][:, ib * 128 : (ib + 1) * 128],
                        wo[:, m * 1024 + fh * 512 : m * 1024 + fh * 512 + 512],
                        start=(m == 0),
                        stop=(m == 3),
                    )
                zsb = pzsb.tile([128, 512], F16, tag="zsb", bufs=2)
                row = (4 * tci + ib) * 128
                if tci == NTC - 1:
                    # tail blocks: ScE is idle after the last exp while the
                    # vector queue drains the last normalize chain; keep the
                    # z triggers on sync so they don't sit between the ScE
                    # copies on the tail critical path
                    nc.scalar.copy(zsb[:], zp[:])
                else:
                    nc.vector.tensor_copy(zsb[:], zp[:])
                nc.sync.dma_start(
                    z[row : row + 128, fh * 512 : fh * 512 + 512], zsb[:]
                )

            def emit_normalize(m, ya, yb, tail=False):
                """ysb[0:64]=ya[0:64]/den_a, ysb[64:128]=yb[0:64]/den_b;
                dens live at PSUM row 64.  For the very last pair (tail
                critical path) the scalar engine and its DMA queue are
                idle after the final exp, so the first copy and the
                row-relocation DMAs run there in parallel with vector."""
                rca = pr.tile([128, 512], F32, tag="rca", bufs=1)
                if tail:
                    nc.scalar.copy(rca[64:65, :], ya[64:65, :])
                else:
                    nc.vector.tensor_copy(rca[64:65, :], ya[64:65, :])
                rcb = pr.tile([128, 512], F32, tag="rcb", bufs=1)
                nc.vector.tensor_copy(rcb[64:65, :], yb[64:65, :])
                rc0 = pr.tile([1, 1024], F32, tag="rc0", bufs=1)
                dq = nc.scalar if tail else nc.sync
                dq.dma_start(rc0[0:1, 0:512], rca[64:65, :])
                dq.dma_start(rc0[0:1, 512:1024], rcb[64:65, :])
                nc.vector.reciprocal_approx_fast(rca[0:1, :], rc0[0:1, 0:512])
                nc.vector.reciprocal_approx_fast(
                    rcb[0:1, :], rc0[0:1, 512:1024]
                )
                rba = pr.tile([128, 512], F32, tag="rba", bufs=2)
                nc.gpsimd.partition_broadcast(rba[0:64, :], rca[0:1, :])
                rbb = pr.tile([128, 512], F32, tag="rbb", bufs=2)
                nc.gpsimd.partition_broadcast(rbb[0:64, :], rcb[0:1, :])
                ytmp = pr.tile([128, 512], F16, tag="ytmp", bufs=1)
                nc.vector.tensor_mul(ytmp[0:64, :], yb[0:64, :], rbb[0:64, :])
                ysb = pysb.tile([128, 512], F16, tag="ysb", name="ysb")
                nc.vector.tensor_mul(ysb[0:64, :], ya[0:64, :], rba[0:64, :])
                dq.dma_start(ysb[64:128, :], ytmp[0:64, :])
                return ysb

            # ---- prologue: the bare minimum to start pair 0 of chunk 0
            # (q0, k0, V block 0); v1-v3 and q1-q3/k1-k3 are produced as
            # in-pair filler, keeping the PE fed at DMA-arrival pace
            xs_of = {0: xs0, 1: emit_x_load(1)}
            qts = {c: [None] * 4 for c in range(NTC)}
            # q0/k0 interleaved in dc-halves: their first halves only need
            # the first two DMA transfers, and a few dummies (on the
            # attention-phase PSUM banks, unused in the prologue) bridge
            # the wait for the second x half so HAM keeps warming
            accq = ps.tile([128, 1024], F32, tag="st", name="acc")
            aq = accq[:, 0:512]
            acck = ps.tile([128, 1024], F32, tag="st", name="acc")
            ak = acck[:, 0:512]
            for acc_, col in ((aq, 0), (ak, 128)):
                for dc in range(4):
                    nc.tensor.matmul(
                        acc_[:],
                        wqk[:, dc * 1024 + col : dc * 1024 + col + 128],
                        xs0[:, dc * 512 : (dc + 1) * 512],
                        start=(dc == 0),
                        stop=False,
                    )
            for _ in range(6):
                wt = pyd.tile([128, 512], F32, tag="ya")
                nc.tensor.matmul(
                    wt[:], ones_f[:, 0:128], ones_f[:], start=True, stop=True
                )
            for acc_, col in ((aq, 0), (ak, 128)):
                for dc in range(4, 8):
                    nc.tensor.matmul(
                        acc_[:],
                        wqk[:, dc * 1024 + col : dc * 1024 + col + 128],
                        xs0[:, dc * 512 : (dc + 1) * 512],
                        start=False,
                        stop=(dc == 7),
                    )
            qt0 = pqt.tile([128, 512], F16, tag="qt", name="qt")
            nc.vector.tensor_copy(qt0[:], aq[:])
            qts[0][0] = qt0
            nc.vector.tensor_copy(kt[0][:, 0:512], ak[:])
            emit_v_group(xs_of[0], 0, 0)

            # out-projection blocks of chunk c-1, drained one per
            # iteration inside chunk c's attention pipeline
            pending = []

            def emit_s(c, m, jb):
                """S block [128 j, 2h x 512 q].  On diagonal blocks the
                q-columns below 128*r are entirely masked, so S only
                computes the suffix (exp and AV skip it too)."""
                qtm = qts[c][m]
                lo = 128 * (jb - 4 * c) if jb >= 4 * c else 0
                st = ps.tile([128, 1024], F32, tag="st", name="st")
                for h in range(2):
                    nc.tensor.matmul(
                        st[:, h * 512 + lo : h * 512 + 512],
                        kt[m][
                            h * 64 : h * 64 + 64,
                            jb * 128 : (jb + 1) * 128,
                        ],
                        qtm[h * 64 : h * 64 + 64, lo:512],
                        start=True,
                        stop=True,
                    )
                return st

            # ---- one global flat pipeline over every (chunk, pair,
            # j-block) iteration; S for iteration i+1 is emitted during
            # iteration i even across pair and chunk boundaries, so the
            # ACT stream never waits on a refill
            allseq = [
                (c, m, jb)
                for c in range(NTC)
                for m in range(4)
                for jb in range(4 * c + 4)
            ]
            ysb_of = {c: [None] * 4 for c in range(NTC)}
            yab = {}
            early_zp = {}
            cidx = 0  # iterations since current chunk's start
            drain_k = 1
            st_next = emit_s(0, 0, 0)
            for idx, (c, m, jb) in enumerate(allseq):
                njb = 4 * c + 4
                if (m, jb) == (0, 0):
                    cidx = 0
                    drain_k = 1
                if jb == 0:
                    ya = pyd.tile([128, 512], F32, tag="ya")
                    yb = pyd.tile([128, 512], F32, tag="yb")
                    yab[(c, m)] = (ya, yb)
                else:
                    ya, yb = yab[(c, m)]
                st = st_next
                pt = ppt.tile([128, 1024], F16, tag="pt", name="pt")
                if jb >= 4 * c:
                    # causal mask: q-columns below 128*r are entirely
                    # masked -> S/exp/AV all skip them (the prefix PSUM
                    # columns were already written by earlier j-blocks);
                    # the q<j cutoff lies inside one 128-col window per
                    # head, multiplied by the precomputed triangle
                    r = jb - 4 * c
                    pt4 = pt[:].rearrange("p (h c) -> p h c", h=2)
                    st4 = st[:].rearrange("p (h c) -> p h c", h=2)
                    if r:
                        nc.scalar.activation(
                            pt4[:, :, 128 * r : 512],
                            st4[:, :, 128 * r : 512],
                            EXP,
                            scale=SCALE,
                        )
                    else:
                        nc.scalar.activation(pt[:], st[:], EXP, scale=SCALE)
                    av_lo = 128 * r
                    masked = True
                else:
                    nc.scalar.activation(pt[:], st[:], EXP, scale=SCALE)
                    av_lo = 0
                    masked = False
                if idx + 1 < len(allseq):
                    st_next = emit_s(*allseq[idx + 1])
                if masked:
                    win = pt4[:, :, 128 * r : 128 * r + 128]
                    nc.vector.tensor_mul(win, win, mtri3)
                first, last = (jb == 0), (jb == njb - 1)
                nc.tensor.matmul(
                    ya[0:96, av_lo:512],
                    vt[jb][:, m * 192 : m * 192 + 96],
                    pt[:, av_lo:512],
                    start=first,
                    stop=last,
                )
                nc.tensor.matmul(
                    yb[0:96, av_lo:512],
                    vt[jb][:, m * 192 + 96 : m * 192 + 192],
                    pt[:, 512 + av_lo : 1024],
                    start=first,
                    stop=last,
                )

                # ---- PE filler: each chunk produces its OWN v blocks and
                # q1-3/k1-3 in its early pairs (only q0/k0 must come from
                # the prior chunk), which shifts movable PE work into the
                # late, ACT-bound chunks; the deferred outproj blocks are
                # spread across the whole chunk for the same reason
                if pending and cidx >= 5 and not masked:
                    # drain only on off-diagonal iterations: the diagonal
                    # suffix-exp -> mask -> AV chain is the tightest slot
                    emit_outproj_block(*pending.pop(0))
                if c == 0 and m == 0:
                    # chunk 0 bootstraps its own V blocks and q1/k1
                    if jb < 3:
                        emit_v_group(xs_of[0], 0, jb + 1)
                    if jb == 2:
                        qts[0][1] = emit_qk_group(xs_of[0], 1, 0)
                        emit_qk_group(xs_of[0], 5, 0)
                elif c == 0 and m < 3:
                    if jb == 0:
                        qts[0][m + 1] = emit_qk_group(xs_of[0], m + 1, 0)
                    elif jb == 1:
                        emit_qk_group(xs_of[0], m + 5, 0)
                if c + 1 < NTC:
                    # next-chunk production: for c>=1 use the early
                    # off-diagonal iterations (the pair-end slots are all
                    # diagonal = the tightest); c0 has no off-diagonal slots
                    f0, f1, f2 = (
                        (njb - 3, njb - 2, njb - 1) if c == 0 else (0, 1, 2)
                    )
                    if jb == f0:
                        qts[c + 1][m] = emit_qk_group(
                            xs_of[c + 1], m, c + 1
                        )
                    elif jb == f1:
                        emit_qk_group(xs_of[c + 1], m + 4, c + 1)
                    elif jb == f2:
                        emit_v_group(xs_of[c + 1], c + 1, m)
                        if m == 0 and c + 2 < NTC:
                            xs_of[c + 2] = emit_x_load(c + 2)
                if last:
                    ysb_of[c][m] = emit_normalize(
                        m, ya, yb, tail=(idx == len(allseq) - 1)
                    )
                    if m == 3:
                        blocks = [
                            (ysb_of[c], c, ib, fh)
                            for ib in range(4)
                            for fh in range(2)
                        ]
                        if c + 1 < NTC:
                            pending = blocks
                        else:
                            # keep the PE warm through the ~7us normalize
                            # drain (HAM re-throttles after ~3.4us idle and
                            # would run the final out-projection at 1.2GHz);
                            # the outproj is chain-blocked anyway, so these
                            # dummies delay nothing
                            for _ in range(26):
                                wt = ps.tile(
                                    [128, 1024], F32, tag="st", name="warm"
                                )
                                nc.tensor.matmul(
                                    wt[:, 0:512],
                                    ones_f[:, 0:128],
                                    ones_f[:],
                                    start=True,
                                    stop=True,
                                )
                            for args in blocks:
                                emit_outproj_block(*args)
                cidx += 1

    nc.finalize()
    _NC_CACHE["nc"] = nc
    return nc


def _in_maps(x, Wqkv, Wout):
    x = np.asarray(x, dtype=np.float32)
    Wqkv = np.asarray(Wqkv, dtype=np.float32)
    Wout = np.asarray(Wout, dtype=np.float32)
    xTs = [np.ascontiguousarray(x[b].T.astype(np.float16)) for b in range(B)]
    maps = []
    for c in range(8):
        b, g = divmod(c, 2)
        qrows = Wqkv[E * g : E * g + E]
        krows = Wqkv[D + E * g : D + E * g + E]
        vrows = Wqkv[2 * D + E * g : 2 * D + E * g + E]
        # interleave [q_m | k_m] blocks of 128 so the first DMA chunk
        # (cols 0:256) is exactly pair 0's weights
        qk = np.concatenate(
            [
                blk
                for mm in range(4)
                for blk in (
                    qrows[mm * 128 : (mm + 1) * 128],
                    krows[mm * 128 : (mm + 1) * 128],
                )
            ],
            axis=0,
        )
        maps.append(
            {
                "xT": xTs[b],
                "wqkT": np.ascontiguousarray(qk.T.astype(np.float16)),
                "wvT": np.ascontiguousarray(vrows.T.astype(np.float16)),
                "woT": np.ascontiguousarray(
                    Wout[:, E * g : E * g + E].T.astype(np.float16)
                ),
            }
        )
    return maps


def _run(x, Wqkv, Wout, trace=False):
    from concourse.bass_utils import run_bass_kernel_spmd

    nc = build()
    res = run_bass_kernel_spmd(
        nc, _in_maps(x, Wqkv, Wout), core_ids=list(range(8)), trace=trace
    )
    out = np.empty((B, T, D), dtype=np.float32)
    for b in range(B):
        out[b] = res.results[2 * b]["z"].astype(np.float32) + res.results[
            2 * b + 1
        ]["z"].astype(np.float32)
    return out, res


def kernel(x, Wqkv, Wout):
    out, _ = _run(x, Wqkv, Wout, trace=False)
    return out


# revision 76
# speedup vs baseline: 1.0249x; 1.0249x over previous
"""Multi-head causal attention on 8 TRN2 NeuronCores.

Problem: x[4,2048,1024] @ Wqkv.T -> 16-head causal attention -> @ Wout.T.

Sharding: core c handles batch b=c//2, head-group g=c%2 (8 heads of 64).
Each core computes qkv for its (batch, head-group) slice, causal attention,
and a partial out-projection over its 512 columns of Wout's input dim.
Host sums the two partials per batch (the all-reduce of the hint).

Per-core layouts (host pre-transposes so every matmul contraction dim lands
on SBUF partitions):
  xT   [1024 d, 2048 t]      wqkT [1024 d, 1024 (q|k)e]
  wvT  [1024 d,  512 e]      woT  [ 512 e, 1024 f]
All tensors are fp16 (PSUM accumulation stays fp32): same 1-row/cycle PE
rate as fp32r but FWL halves LDWEIGHTS, DMA bytes halve, and the PE power
draw stays under the SW-throttle threshold that cost fp32r ~75us of K=4/8
clock-gating.  Simulated end-to-end fp16 error: 5.7e-4 rel (gate: 2e-2).

Schedule: a single flat software pipeline over (pair, j-block) iterations.
S for iteration i+1 is emitted before AV of iteration i (crossing pair
boundaries, so ACT never waits on a pair refill); next-chunk QKV production
groups and the previous chunk's out-projection blocks are sprinkled one
per iteration as PE filler; weights/x arrive via 5 consolidated strided
DMAs (sync-queue trigger rate, not bandwidth, gated the old prologue).
S head-pairs run concurrently on row-groups 0:63/64:127 (auto
tile_position from the 64-partition APs).
"""

import sys

sys.path.insert(0, "/opt/trn_rl_repo")

import numpy as np

B, T, D, H = 4, 2048, 1024, 16
E = 512  # per-core head width (8 heads x 64)
ND = 8  # d chunks of 128
NTC = 4  # t chunks of 512
SCALE = 0.125  # 1/sqrt(64)

_NC_CACHE = {}


def build():
    if "nc" in _NC_CACHE:
        return _NC_CACHE["nc"]
    import concourse.bacc as bacc
    import concourse.mybir as mybir
    import concourse.tile as tile

    F32 = mybir.dt.float32
    F16 = mybir.dt.float16
    EXP = mybir.ActivationFunctionType.Exp

    nc = bacc.Bacc("TRN2", target_bir_lowering=False, debug=False, num_devices=8)
    xT = nc.declare_dram_parameter("xT", [D, T], F16, isOutput=False)
    wqkT = nc.declare_dram_parameter("wqkT", [D, 2 * E], F16, isOutput=False)
    wvT = nc.declare_dram_parameter("wvT", [D, E], F16, isOutput=False)
    woT = nc.declare_dram_parameter("woT", [E, D], F16, isOutput=False)
    z = nc.declare_dram_parameter("z", [T, D], F16, isOutput=True)

    with tile.TileContext(nc) as tc:
        with (
            tc.tile_pool(name="pw", bufs=1) as pw,
            tc.tile_pool(name="px", bufs=3) as px,
            tc.tile_pool(name="pkt", bufs=4) as pkt,
            tc.tile_pool(name="pqt", bufs=8) as pqt,
            tc.tile_pool(name="pv", bufs=16) as pv,
            tc.tile_pool(name="ppt", bufs=2) as ppt,
            tc.tile_pool(name="pr", bufs=2) as pr,
            tc.tile_pool(name="pysb", bufs=8) as pysb,
            tc.tile_pool(name="pzsb", bufs=1) as pzsb,
            tc.tile_pool(name="pone", bufs=1) as pone,
            tc.tile_pool(name="ps", bufs=2, space="PSUM") as ps,
            tc.tile_pool(name="pyd", bufs=2, space="PSUM") as pyd,
        ):
            # ---- consolidated input DMAs (one trigger each; the sync
            # queue issues triggers at only ~0.65us apiece)
            wqk = pw.tile([128, ND * 2 * E], F16, tag="wqk")
            wqk3 = wqk[:].rearrange("p (dc e) -> p dc e", dc=ND)
            xs0 = px.tile([128, ND * 512], F16, tag="x", name="xs")
            wv = pw.tile([128, ND * E], F16, tag="wv")
            wo = pw.tile([128, 4 * D], F16, tag="wo")

            # transfers alternate across BOTH hwdge trigger queues
            # (SP + Activation) and arrive in order of first use: the host
            # stores wqkT columns as [q0,k0,q1,k1,...] so the first 512KB
            # chunk is exactly pair 0's q/k weights; the ~320GB/s aggregate
            # DMA bandwidth is the prologue's floor, so byte order is
            # everything
            xs03 = xs0[:].rearrange("p (dc t) -> p dc t", dc=ND)
            xT3 = xT[:, 0:512].rearrange("(dc p) t -> p dc t", p=128)
            wqkT3 = wqkT[:].rearrange("(dc p) e -> p dc e", p=128)
            wv3 = wv[:].rearrange("p (dc e) -> p dc e", dc=ND)
            wvT3 = wvT[:].rearrange("(dc p) e -> p dc e", p=128)
            nc.sync.dma_start(wqk3[:, :, 0:256], wqkT3[:, :, 0:256])
            nc.scalar.dma_start(xs03[:, 0:4, :], xT3[:, 0:4, :])
            nc.sync.dma_start(xs03[:, 4:8, :], xT3[:, 4:8, :])
            nc.scalar.dma_start(wv3[:, 0:4, :], wvT3[:, 0:4, :])
            nc.sync.dma_start(wv3[:, 4:8, :], wvT3[:, 4:8, :])
            nc.scalar.dma_start(wqk3[:, :, 256:512], wqkT3[:, :, 256:512])
            nc.sync.dma_start(wqk3[:, :, 512:768], wqkT3[:, :, 512:768])
            nc.scalar.dma_start(wqk3[:, :, 768:1024], wqkT3[:, :, 768:1024])
            nc.sync.dma_start(
                wo[:].rearrange("p (m f) -> p m f", m=4),
                woT[:].rearrange("(m p) f -> p m f", p=128),
            )

            # per-head filler block for the AV stationary: [ones(32)|zeros(32)]
            ones_f = pone.tile([128, 512], F16, tag="onef")
            of4 = ones_f[:].rearrange("p (hh c) -> p hh c", hh=8)
            nc.gpsimd.memset(of4[:, :, 0:32], 1.0)
            nc.gpsimd.memset(of4[:, :, 32:64], 0.0)

            # 0/1 upper-triangle (keep c>=j) mask for the causal window;
            # applied as a DVE multiply so gpsimd runs only its
            # partition_broadcast library (an affine_select/broadcast mix
            # thrashes the gpsimd custom-op library, ~6us per swap)
            mtri = pone.tile([128, 128], F16, tag="mtri")
            nc.vector.memset(mtri[:], 1.0)
            nc.gpsimd.affine_select(
                out=mtri[:],
                in_=mtri[:],
                compare_op=mybir.AluOpType.is_ge,
                fill=0.0,
                base=0,
                pattern=[[1, 128]],
                channel_multiplier=-1,
            )
            mtri3 = (
                mtri[:]
                .rearrange("p (o c) -> p o c", o=1)
                .broadcast_to((128, 2, 128))
            )

            # a few dummy matmuls on the ones tile bridge the PE from the
            # preamble to the first DMA arrival so the HAM clock-gate
            # warmup (~3.4us of sustained activity) starts immediately
            for _ in range(3):
                warm = ps.tile([128, 1024], F32, tag="st", name="warm")
                nc.tensor.matmul(
                    warm[:, 0:512],
                    ones_f[:, 0:128],
                    ones_f[:],
                    start=True,
                    stop=True,
                )

            # persistent K^T [e,t] tiles; pair m = heads 2m / 2m+1 at
            # partition rows 0:64 / 64:128
            kt = [
                pkt.tile([128, T], F16, tag="kt", name=f"kt{i}")
                for i in range(4)
            ]
            vt = [None] * 16  # V tiles per 128-row t-block

            def emit_x_load(tci):
                t_ = px.tile([128, ND * 512], F16, tag="x", name="xs")
                t0 = tci * 512
                nc.sync.dma_start(
                    t_[:].rearrange("p (dc t) -> p dc t", dc=ND),
                    xT[:, t0 : t0 + 512].rearrange(
                        "(dc p) t -> p dc t", p=128
                    ),
                )
                return t_

            def emit_qk_group(xs, m, tci):
                """m 0..3: Q chunk -> returns qt tile; 4..7: K chunk.
                wqk column order per dc is [q0,k0,q1,k1,...] (128 each)."""
                col = (m % 4) * 256 + (128 if m >= 4 else 0)
                acc = ps.tile([128, 1024], F32, tag="st", name="acc")
                acc = acc[:, 0:512]
                for dc in range(ND):
                    nc.tensor.matmul(
                        acc[:],
                        wqk[:, dc * 1024 + col : dc * 1024 + col + 128],
                        xs[:, dc * 512 : (dc + 1) * 512],
                        start=(dc == 0),
                        stop=(dc == ND - 1),
                    )
                if m < 4:
                    t_ = pqt.tile([128, 512], F16, tag="qt", name="qt")
                    nc.vector.tensor_copy(t_[:], acc[:])
                    return t_
                t0 = tci * 512
                nc.vector.tensor_copy(kt[m - 4][:, t0 : t0 + 512], acc[:])
                return None

            def emit_v_group(xs, tci, ts):
                """V tile layout per head: [V_h(64) | ones(32)] -> AV lhsT
                slices are 96 cols, putting Y at PSUM rows 0:64 and the
                denominator at 64:96."""
                jb = 4 * tci + ts
                acc = ps.tile([128, 1024], F32, tag="st", name="vacc")
                acc = acc[:, 0:512]
                for dc in range(ND):
                    nc.tensor.matmul(
                        acc[:],
                        xs[:, dc * 512 + ts * 128 : dc * 512 + (ts + 1) * 128],
                        wv[:, dc * 512 : (dc + 1) * 512],
                        start=(dc == 0),
                        stop=(dc == ND - 1),
                    )
                t_ = pv.tile([128, 768], F16, tag="v", name="vt")
                t4 = t_[:].rearrange("p (hh c) -> p hh c", hh=8)
                a4 = acc[:].rearrange("p (hh c) -> p hh c", hh=8)
                nc.vector.tensor_copy(t4[:, :, 0:64], a4[:])
                nc.vector.tensor_copy(t4[:, :, 64:96], of4[:, :, 0:32])
                vt[jb] = t_

            def emit_outproj_block(ysbs, tci, ib, fh):
                zp = ps.tile([128, 1024], F32, tag="st", name="zp")
                zp = zp[:, 0:512]
                for m in range(4):
                    nc.tensor.matmul(
                        zp[:],
                        ysbs[m][:, ib * 128 : (ib + 1) * 128],
                        wo[:, m * 1024 + fh * 512 : m * 1024 + fh * 512 + 512],
                        start=(m == 0),
                        stop=(m == 3),
                    )
                zsb = pzsb.tile([128, 512], F16, tag="zsb", bufs=2)
                row = (4 * tci + ib) * 128
                if tci == NTC - 1:
                    # tail blocks: ScE is idle after the last exp while the
                    # vector queue drains the last normalize chain; keep the
                    # z triggers on sync so they don't sit between the ScE
                    # copies on the tail critical path
                    nc.scalar.copy(zsb[:], zp[:])
                else:
                    nc.vector.tensor_copy(zsb[:], zp[:])
                nc.sync.dma_start(
                    z[row : row + 128, fh * 512 : fh * 512 + 512], zsb[:]
                )

            def emit_normalize(m, ya, yb, tail=False):
                """ysb[0:64]=ya[0:64]/den_a, ysb[64:128]=yb[0:64]/den_b;
                dens live at PSUM row 64.  For the very last pair (tail
                critical path) the scalar engine and its DMA queue are
                idle after the final exp, so the first copy and the
                row-relocation DMAs run there in parallel with vector."""
                rca = pr.tile([128, 512], F32, tag="rca", bufs=1)
                if tail:
                    nc.scalar.copy(rca[64:65, :], ya[64:65, :])
                else:
                    nc.vector.tensor_copy(rca[64:65, :], ya[64:65, :])
                rcb = pr.tile([128, 512], F32, tag="rcb", bufs=1)
                nc.vector.tensor_copy(rcb[64:65, :], yb[64:65, :])
                rc0 = pr.tile([1, 1024], F32, tag="rc0", bufs=1)
                dq = nc.scalar if tail else nc.sync
                dq.dma_start(rc0[0:1, 0:512], rca[64:65, :])
                dq.dma_start(rc0[0:1, 512:1024], rcb[64:65, :])
                nc.vector.reciprocal_approx_fast(rca[0:1, :], rc0[0:1, 0:512])
                nc.vector.reciprocal_approx_fast(
                    rcb[0:1, :], rc0[0:1, 512:1024]
                )
                rba = pr.tile([128, 512], F32, tag="rba", bufs=2)
                nc.gpsimd.partition_broadcast(rba[0:64, :], rca[0:1, :])
                rbb = pr.tile([128, 512], F32, tag="rbb", bufs=2)
                nc.gpsimd.partition_broadcast(rbb[0:64, :], rcb[0:1, :])
                ytmp = pr.tile([128, 512], F16, tag="ytmp", bufs=1)
                nc.vector.tensor_mul(ytmp[0:64, :], yb[0:64, :], rbb[0:64, :])
                ysb = pysb.tile([128, 512], F16, tag="ysb", name="ysb")
                nc.vector.tensor_mul(ysb[0:64, :], ya[0:64, :], rba[0:64, :])
                dq.dma_start(ysb[64:128, :], ytmp[0:64, :])
                return ysb

            # ---- prologue: the bare minimum to start pair 0 of chunk 0
            # (q0, k0, V block 0); v1-v3 and q1-q3/k1-k3 are produced as
            # in-pair filler, keeping the PE fed at DMA-arrival pace
            xs_of = {0: xs0, 1: emit_x_load(1)}
            qts = {c: [None] * 4 for c in range(NTC)}
            # q0/k0 interleaved in dc-halves: their first halves only need
            # the first two DMA transfers, and a few dummies (on the
            # attention-phase PSUM banks, unused in the prologue) bridge
            # the wait for the second x half so HAM keeps warming
            accq = ps.tile([128, 1024], F32, tag="st", name="acc")
            aq = accq[:, 0:512]
            acck = ps.tile([128, 1024], F32, tag="st", name="acc")
            ak = acck[:, 0:512]
            for acc_, col in ((aq, 0), (ak, 128)):
                for dc in range(4):
                    nc.tensor.matmul(
                        acc_[:],
                        wqk[:, dc * 1024 + col : dc * 1024 + col + 128],
                        xs0[:, dc * 512 : (dc + 1) * 512],
                        start=(dc == 0),
                        stop=False,
                    )
            for _ in range(6):
                wt = pyd.tile([128, 512], F32, tag="ya")
                nc.tensor.matmul(
                    wt[:], ones_f[:, 0:128], ones_f[:], start=True, stop=True
                )
            for acc_, col in ((aq, 0), (ak, 128)):
                for dc in range(4, 8):
                    nc.tensor.matmul(
                        acc_[:],
                        wqk[:, dc * 1024 + col : dc * 1024 + col + 128],
                        xs0[:, dc * 512 : (dc + 1) * 512],
                        start=False,
                        stop=(dc == 7),
                    )
            qt0 = pqt.tile([128, 512], F16, tag="qt", name="qt")
            nc.vector.tensor_copy(qt0[:], aq[:])
            qts[0][0] = qt0
            nc.vector.tensor_copy(kt[0][:, 0:512], ak[:])
            emit_v_group(xs_of[0], 0, 0)

            # out-projection blocks of chunk c-1, drained one per
            # iteration inside chunk c's attention pipeline
            pending = []

            def emit_s(c, m, jb):
                """S block [128 j, 2h x 512 q].  On diagonal blocks the
                q-columns below 128*r are entirely masked, so S only
                computes the suffix (exp and AV skip it too)."""
                qtm = qts[c][m]
                lo = 128 * (jb - 4 * c) if jb >= 4 * c else 0
                st = ps.tile([128, 1024], F32, tag="st", name="st")
                for h in range(2):
                    nc.tensor.matmul(
                        st[:, h * 512 + lo : h * 512 + 512],
                        kt[m][
                            h * 64 : h * 64 + 64,
                            jb * 128 : (jb + 1) * 128,
                        ],
                        qtm[h * 64 : h * 64 + 64, lo:512],
                        start=True,
                        stop=True,
                    )
                return st

            # ---- one global flat pipeline over every (chunk, pair,
            # j-block) iteration; S for iteration i+1 is emitted during
            # iteration i even across pair and chunk boundaries, so the
            # ACT stream never waits on a refill
            allseq = [
                (c, m, jb)
                for c in range(NTC)
                for m in range(4)
                for jb in range(4 * c + 4)
            ]
            ysb_of = {c: [None] * 4 for c in range(NTC)}
            yab = {}
            early_zp = {}
            cidx = 0  # iterations since current chunk's start
            drain_k = 1
            st_next = emit_s(0, 0, 0)
            for idx, (c, m, jb) in enumerate(allseq):
                njb = 4 * c + 4
                if (m, jb) == (0, 0):
                    cidx = 0
                    drain_k = 1
                if jb == 0:
                    ya = pyd.tile([128, 512], F32, tag="ya")
                    yb = pyd.tile([128, 512], F32, tag="yb")
                    yab[(c, m)] = (ya, yb)
                else:
                    ya, yb = yab[(c, m)]
                st = st_next
                pt = ppt.tile([128, 1024], F16, tag="pt", name="pt")
                if jb >= 4 * c:
                    # causal mask: q-columns below 128*r are entirely
                    # masked -> S/exp/AV all skip them (the prefix PSUM
                    # columns were already written by earlier j-blocks);
                    # the q<j cutoff lies inside one 128-col window per
                    # head, multiplied by the precomputed triangle
                    r = jb - 4 * c
                    pt4 = pt[:].rearrange("p (h c) -> p h c", h=2)
                    st4 = st[:].rearrange("p (h c) -> p h c", h=2)
                    if r:
                        nc.scalar.activation(
                            pt4[:, :, 128 * r : 512],
                            st4[:, :, 128 * r : 512],
                            EXP,
                            scale=SCALE,
                        )
                    else:
                        nc.scalar.activation(pt[:], st[:], EXP, scale=SCALE)
                    av_lo = 128 * r
                    masked = True
                else:
                    nc.scalar.activation(pt[:], st[:], EXP, scale=SCALE)
                    av_lo = 0
                    masked = False
                if idx + 1 < len(allseq):
                    st_next = emit_s(*allseq[idx + 1])
                if masked:
                    win = pt4[:, :, 128 * r : 128 * r + 128]
                    nc.vector.tensor_mul(win, win, mtri3)
                first, last = (jb == 0), (jb == njb - 1)
                nc.tensor.matmul(
                    ya[0:96, av_lo:512],
                    vt[jb][:, m * 192 : m * 192 + 96],
                    pt[:, av_lo:512],
                    start=first,
                    stop=last,
                )
                nc.tensor.matmul(
                    yb[0:96, av_lo:512],
                    vt[jb][:, m * 192 + 96 : m * 192 + 192],
                    pt[:, 512 + av_lo : 1024],
                    start=first,
                    stop=last,
                )

                # ---- PE filler: each chunk produces its OWN v blocks and
                # q1-3/k1-3 in its early pairs (only q0/k0 must come from
                # the prior chunk), which shifts movable PE work into the
                # late, ACT-bound chunks; the deferred outproj blocks are
                # spread across the whole chunk for the same reason
                if pending and cidx >= 5 and not masked:
                    # drain only on off-diagonal iterations: the diagonal
                    # suffix-exp -> mask -> AV chain is the tightest slot
                    emit_outproj_block(*pending.pop(0))
                if c == 0 and m == 0:
                    # chunk 0 bootstraps its own V blocks and q1/k1
                    if jb < 3:
                        emit_v_group(xs_of[0], 0, jb + 1)
                    if jb == 2:
                        qts[0][1] = emit_qk_group(xs_of[0], 1, 0)
                        emit_qk_group(xs_of[0], 5, 0)
                elif c == 0 and m < 3:
                    if jb == 0:
                        qts[0][m + 1] = emit_qk_group(xs_of[0], m + 1, 0)
                    elif jb == 1:
                        emit_qk_group(xs_of[0], m + 5, 0)
                if c + 1 < NTC:
                    if jb == njb - 3:
                        qts[c + 1][m] = emit_qk_group(
                            xs_of[c + 1], m, c + 1
                        )
                    elif jb == njb - 2:
                        emit_qk_group(xs_of[c + 1], m + 4, c + 1)
                    elif jb == njb - 1:
                        emit_v_group(xs_of[c + 1], c + 1, m)
                        if m == 0 and c + 2 < NTC:
                            xs_of[c + 2] = emit_x_load(c + 2)
                if last:
                    ysb_of[c][m] = emit_normalize(
                        m, ya, yb, tail=(idx == len(allseq) - 1)
                    )
                    if m == 3:
                        blocks = [
                            (ysb_of[c], c, ib, fh)
                            for ib in range(4)
                            for fh in range(2)
                        ]
                        if c + 1 < NTC:
                            pending = blocks
                        else:
                            # keep the PE warm through the ~7us normalize
                            # drain (HAM re-throttles after ~3.4us idle and
                            # would run the final out-projection at 1.2GHz);
                            # the outproj is chain-blocked anyway, so these
                            # dummies delay nothing
                            for _ in range(26):
                                wt = ps.tile(
                                    [128, 1024], F32, tag="st", name="warm"
                                )
                                nc.tensor.matmul(
                                    wt[:, 0:512],
                                    ones_f[:, 0:128],
                                    ones_f[:],
                                    start=True,
                                    stop=True,
                                )
                            for args in blocks:
                                emit_outproj_block(*args)
                cidx += 1

    nc.finalize()
    _NC_CACHE["nc"] = nc
    return nc


def _in_maps(x, Wqkv, Wout):
    x = np.asarray(x, dtype=np.float32)
    Wqkv = np.asarray(Wqkv, dtype=np.float32)
    Wout = np.asarray(Wout, dtype=np.float32)
    xTs = [np.ascontiguousarray(x[b].T.astype(np.float16)) for b in range(B)]
    maps = []
    for c in range(8):
        b, g = divmod(c, 2)
        qrows = Wqkv[E * g : E * g + E]
        krows = Wqkv[D + E * g : D + E * g + E]
        vrows = Wqkv[2 * D + E * g : 2 * D + E * g + E]
        # interleave [q_m | k_m] blocks of 128 so the first DMA chunk
        # (cols 0:256) is exactly pair 0's weights
        qk = np.concatenate(
            [
                blk
                for mm in range(4)
                for blk in (
                    qrows[mm * 128 : (mm + 1) * 128],
                    krows[mm * 128 : (mm + 1) * 128],
                )
            ],
            axis=0,
        )
        maps.append(
            {
                "xT": xTs[b],
                "wqkT": np.ascontiguousarray(qk.T.astype(np.float16)),
                "wvT": np.ascontiguousarray(vrows.T.astype(np.float16)),
                "woT": np.ascontiguousarray(
                    Wout[:, E * g : E * g + E].T.astype(np.float16)
                ),
            }
        )
    return maps


def _run(x, Wqkv, Wout, trace=False):
    from concourse.bass_utils import run_bass_kernel_spmd

    nc = build()
    res = run_bass_kernel_spmd(
        nc, _in_maps(x, Wqkv, Wout), core_ids=list(range(8)), trace=trace
    )
    out = np.empty((B, T, D), dtype=np.float32)
    for b in range(B):
        out[b] = res.results[2 * b]["z"].astype(np.float32) + res.results[
            2 * b + 1
        ]["z"].astype(np.float32)
    return out, res


def kernel(x, Wqkv, Wout):
    out, _ = _run(x, Wqkv, Wout, trace=False)
    return out
